# revision 35
# baseline (speedup 1.0000x reference)
"""EquivariantGNN message-passing kernel for Trainium2 (8 NeuronCores, SPMD).

Math (matches the reference):
  x   = [pos | onehot(z)] @ [[I3,0],[0,emb]]          (rank-8 node features)
  logits[e,h] = 0.25 * q[dst]. (k[src]+eb)
              = sum_{i,j} x8[dst][i]*srcext[e][j] * Bvec[(i,j),h]
  w = exp(logits)  (no max subtraction needed; logits are O(10))
  den[n,h] = sum_{dst(e)=n} w
  U[n,(h,j)] = sum_{dst(e)=n} w[e,h]*srcext[e,j]      (96 values per node)
  agg[n,h,:] = (U[n,h,:]/den[n,h]) @ Wve12[:,h-slice]  (ve folded per NODE)
  out = agg @ Wo + x ; S = sum_n relu(out) ; answer = (S @ lin_w)/N + lin_b

Device strategy per core: edges sorted by dst, 128-edge blocks each fully inside
one 128-node tile.  Host ships, per edge, the 96-dim kron row
x8[dst] (x) srcext (lhsT layout), plus srcext row-major.  Per block one tiny
matmul (rhs [96,8]) yields the logits; ACT exponentiates; DVE forms
[w | w (x) srcext] (104 wide); a onehot(localdst) fp8 matmul
scatter-accumulates it into a per-tile PSUM accumulator.  The tile epilogue
normalizes U by den, transposes, and applies the host-folded
(Wve12 blockdiag) @ Wo plus the x8 @ (J8 @ ... residual), relu, and
accumulates the node-sum S via a ones-matmul.
"""

import math
import os
import sys

import numpy as np

for _p in ("/opt/trn_rl_repo", "/root/.axon_site/_ro/trn_rl_repo"):
    if os.path.isdir(_p) and _p not in sys.path:
        sys.path.insert(0, _p)

P = 128
DIM = 128
H, DH = 8, 16
DE = 4
NF12 = 12   # srcext = [ea(4) | x8[src](8)]
NKRON = 96  # U payload width: H * NF12
NLG = 44    # logit lhsT rows: pos-kron(36) + dst-type term(8)
NU = 8 + NKRON  # scatter payload: [w(8) | w (x) srcext (96)]
N_CORES = 8
GB = 8       # blocks per processing group
GCHUNK = 32  # blocks per gather chunk

# test-harness knobs (the grading harness just calls kernel() with defaults)
PROFILE = False
TRACE_CORES = None
LAST_RESULT = None  # BassKernelResults of the last run (for profiling)
_PROG_CACHE = {}


# ---------------------------------------------------------------- host prep
def _host_prep(pos, edge_attr, emb, Wq, Wk, Wv, We, Wo, z, edge_index):
    f32 = np.float32
    N = pos.shape[0]
    NT = emb.shape[0]
    ntiles = (N + P - 1) // P
    npad = ntiles * P

    z = np.asarray(z).astype(np.int64)
    src = np.asarray(edge_index[0]).astype(np.int64)
    dst = np.asarray(edge_index[1]).astype(np.int64)
    E = src.shape[0]

    onehot = np.zeros((N, NT), f32)
    onehot[np.arange(N), z] = 1.0
    x8 = np.concatenate([np.asarray(pos, f32), onehot], axis=1)  # [N, 8]
    x8p = np.zeros((npad, 8), f32)
    x8p[:N] = x8

    # rank-8 weight factors
    Wq8 = np.vstack([Wq[:3], emb @ Wq[3:]]).astype(f32)  # [8,128]
    Wk8 = np.vstack([Wk[:3], emb @ Wk[3:]]).astype(f32)
    Wv8 = np.vstack([Wv[:3], emb @ Wv[3:]]).astype(f32)
    # srcext rows = [ea(4); x8src(8)]:  ke = srcext @ [[We],[Wk8]]
    Wke12 = np.vstack([We, Wk8]).astype(f32)   # [12,128]
    Wve12 = np.vstack([We, Wv8]).astype(f32)   # [12,128]

    # bilinear logits: logits[e,h] = sum_{i,j} x8dst[i]*srcext[j]*Bvec[(i,j),h]
    Bvec = np.zeros((NKRON, H), f32)
    for h in range(H):
        Bh = 0.25 * (Wq8[:, h * DH:(h + 1) * DH]
                     @ Wke12[:, h * DH:(h + 1) * DH].T)  # [8,12]
        Bvec[:, h] = Bh.reshape(NKRON)

    # U[(h,j)] -> out:  WblkWo[(h,j), d'] = sum_d Wve12[j, h*16+d] * Wo[h*16+d, d']
    Wo32 = np.asarray(Wo, f32)
    WblkWo = np.zeros((NKRON, DIM), f32)
    for h in range(H):
        WblkWo[h * NF12:(h + 1) * NF12] = (
            Wve12[:, h * DH:(h + 1) * DH] @ Wo32[h * DH:(h + 1) * DH])

    J8 = np.zeros((8, DIM), f32)  # x = x8 @ J8
    J8[0:3, 0:3] = np.eye(3, dtype=f32)
    J8[3:8, 3:DIM] = emb

    # ---- sort edges by dst, split into per-node-tile runs
    perm = np.argsort(dst, kind="stable")
    src_s, dst_s = src[perm], dst[perm]
    ea_s = np.asarray(edge_attr, f32)[perm]
    tile_of_edge = dst_s // P
    starts = np.searchsorted(tile_of_edge, np.arange(ntiles))
    ends = np.searchsorted(tile_of_edge, np.arange(ntiles) + 1)
    ecnt = ends - starts
    nb = np.maximum(1, (ecnt + P - 1) // P)  # blocks per real tile

    # per-edge srcext + kron rows
    sext = np.empty((E, NF12), f32)
    sext[:, 0:DE] = ea_s
    sext[:, DE:NF12] = x8[src_s]
    kron = (x8[dst_s][:, :, None] * sext[:, None, :]).reshape(E, NKRON)

    # ---- uniform schedule across cores: pad tile list to multiple of 8,
    # sort by block count desc, deal groups of 8 (one tile per core),
    # pad each group to the group max -> identical counts on every core.
    ntiles_tot = ((ntiles + N_CORES - 1) // N_CORES) * N_CORES
    nb_all = np.concatenate([nb, np.ones(ntiles_tot - ntiles, np.int64)])
    order = np.argsort(-nb_all, kind="stable")
    TS = ntiles_tot // N_CORES  # tiles per core
    counts = [int(nb_all[order[8 * k]]) for k in range(TS)]  # group max
    counts[-1] += (-sum(counts)) % GB  # block count multiple of the group size
    C = int(sum(counts))

    import ml_dtypes

    srcfac = np.zeros((N_CORES, C, P, NKRON), ml_dtypes.bfloat16)
    se12 = np.zeros((N_CORES, P, C, NF12), ml_dtypes.bfloat16)
    ohmat = np.zeros((N_CORES, C, P, P), ml_dtypes.float8_e4m3fn)  # onehot(localdst)
    xT8c = np.zeros((N_CORES, 8, TS * P), f32)

    offs = np.concatenate([[0], np.cumsum(counts)])
    for k in range(TS):
        for j in range(N_CORES):
            t = int(order[8 * k + j])
            if t >= ntiles:
                continue  # dummy tile: all-dummy blocks, zero xT8c
            xT8c[j, :, k * P:(k + 1) * P] = x8p[t * P:(t + 1) * P].T
            e0, e1 = int(starts[t]), int(ends[t])
            ne = e1 - e0
            if ne == 0:
                continue
            c0 = int(offs[k])
            flat = np.arange(ne)
            cc = c0 + flat // P
            pp = flat % P
            srcfac[j, cc, pp, :] = kron[e0:e1]
            se12[j, pp, cc, :] = sext[e0:e1]
            ohmat[j, cc, pp, dst_s[e0:e1] - t * P] = 1.0

    ident = np.eye(P, dtype=f32)
    ones = np.ones((P, 1), f32)

    # device layouts
    srcfacT = np.ascontiguousarray(
        srcfac.transpose(0, 3, 1, 2)).reshape(N_CORES, NKRON, C * P)
    ohmatd = np.ascontiguousarray(ohmat.transpose(0, 2, 1, 3))  # [j, P, C, P]

    WblkWoJ = np.vstack([WblkWo, J8])  # residual folded as 8 extra lhsT rows
    shared = dict(rhs96=Bvec.astype(ml_dtypes.bfloat16),
                  WblkWo=WblkWoJ.astype(ml_dtypes.bfloat16), ident=ident,
                  ones=ones.astype(ml_dtypes.bfloat16))
    percore = dict(srcfacT=srcfacT, se12=se12, ohmat=ohmatd,
                   xT8c=xT8c.astype(ml_dtypes.bfloat16))
    meta = dict(counts=counts, C=C, TS=TS, npad=npad, N=N, E=E)
    return shared, percore, meta


# ---------------------------------------------------------------- device code
def _build_program(counts, C, TS, npad):
    import concourse.bacc as bacc
    import concourse.bass as bass
    import concourse.tile as tile
    from concourse import mybir
    from concourse._compat import with_exitstack  # noqa: F401

    f32 = mybir.dt.float32
    bf16 = mybir.dt.bfloat16
    fp8 = mybir.dt.float8e4

    nc = bacc.Bacc("TRN2", target_bir_lowering=False, debug=False,
                   enable_asserts=False, num_devices=N_CORES)

    srcfacT_in = nc.dram_tensor("srcfacT", [NKRON, C * P], bf16,
                                kind="ExternalInput").ap()
    se12_in = nc.dram_tensor("se12", [P, C, NF12], bf16,
                             kind="ExternalInput").ap()
    ohmat_in = nc.dram_tensor("ohmat", [P, C, P], fp8, kind="ExternalInput").ap()
    xT8c_in = nc.dram_tensor("xT8c", [8, TS * P], bf16, kind="ExternalInput").ap()
    rhs96_in = nc.dram_tensor("rhs96", [NKRON, H], bf16,
                              kind="ExternalInput").ap()
    WblkWo_in = nc.dram_tensor("WblkWo", [NKRON + 8, DIM], bf16,
                               kind="ExternalInput").ap()
    ident_in = nc.dram_tensor("ident", [P, P], f32, kind="ExternalInput").ap()
    ones_in = nc.dram_tensor("ones", [P, 1], bf16, kind="ExternalInput").ap()
    S_out = nc.dram_tensor("S_out", [1, 4 * DIM], f32, kind="ExternalOutput").ap()

    with tile.TileContext(nc) as tc:
        with (
            tc.tile_pool(name="const", bufs=1) as constp,
            tc.tile_pool(name="chunks", bufs=6) as chunkp,
            tc.tile_pool(name="blk", bufs=6) as blkp,
            tc.tile_pool(name="psmain", bufs=2, space="PSUM") as psmainp,
            tc.tile_pool(name="psmisc", bufs=1, space="PSUM") as psmiscp,
            tc.tile_pool(name="psacc", bufs=3, space="PSUM") as psaccp,
            tc.tile_pool(name="psS", bufs=1, space="PSUM") as psSp,
        ):
            # HAM warmup: ~5us of back-to-back matmuls on a memset tile (no
            # DMA dependency) so the PE clock un-throttles to 2.4 GHz while
            # the first chunks stream in.
            warm_sb = constp.tile([P, 2 * P], bf16, tag="warmsrc")
            nc.gpsimd.memset(warm_sb[:], 0.5)
            pswarm = psmiscp.tile([P, DIM], f32, tag="T")
            for _ in range(16):
                nc.tensor.matmul(pswarm[:], lhsT=warm_sb[:, 0:P],
                                 rhs=warm_sb[:, P:2 * P], start=True, stop=True)

            # chunk schedule: small prologue chunks so compute starts early
            bounds = [0]
            for nxt in (8, 24, 56):
                if nxt < C:
                    bounds.append(nxt)
            while bounds[-1] + GCHUNK < C:
                bounds.append(bounds[-1] + GCHUNK)
            bounds.append(C)
            cidx_of = {}
            for ci in range(len(bounds) - 1):
                for g in range(bounds[ci], bounds[ci + 1]):
                    cidx_of[g] = ci

            chunks = {}

            def load_chunk(ci):
                g0, g1 = bounds[ci], bounds[ci + 1]
                gn = g1 - g0
                st = chunkp.tile([NKRON, GCHUNK * P], bf16, tag="srcT")
                ohc = chunkp.tile([P, GCHUNK, P], fp8, tag="ohc")
                nc.sync.dma_start(out=st[:, :gn * P],
                                  in_=srcfacT_in[:, g0 * P:(g0 + gn) * P])
                nc.sync.dma_start(out=ohc[:, :gn, :],
                                  in_=ohmat_in[:, g0:g0 + gn, :])
                chunks[ci] = (st, ohc, g0)

            # critical-path loads first: the logit weights, srcext rows and
            # the first two edge chunks; remaining constants go out on the
            # (otherwise idle) GpSimd DMA queue.
            load_chunk(0)
            rhs96_sb = constp.tile_from(rhs96_in)
            SE_SPLIT = min(56, C)
            se12a_sb = constp.tile([P, SE_SPLIT, NF12], bf16, tag="se12a")
            nc.sync.dma_start(out=se12a_sb[:], in_=se12_in[:, 0:SE_SPLIT, :])
            load_chunk(1)
            if len(bounds) > 3:
                load_chunk(2)
            se12b_sb = constp.tile([P, C - SE_SPLIT, NF12], bf16, tag="se12b")
            nc.sync.dma_start(out=se12b_sb[:], in_=se12_in[:, SE_SPLIT:C, :])
            pool_eng = mybir.EngineType.Pool
            WblkWo_sb = constp.tile_from(WblkWo_in, forced_dma_engine=pool_eng)
            xT8c_sb = constp.tile_from(xT8c_in, forced_dma_engine=pool_eng)
            ident_sb = constp.tile_from(ident_in, forced_dma_engine=pool_eng)
            identb_sb = constp.tile_from(ident_in, dtype=mybir.dt.bfloat16,
                                         forced_dma_engine=pool_eng,
                                         force_copy=True)
            ones_sb = constp.tile_from(ones_in, forced_dma_engine=pool_eng)

            psS = psSp.tile([1, 4 * DIM], f32, tag="S")
            hall = constp.tile([P, TS * DIM], bf16, tag="hall")
            nslice = (TS + 3) // 4

            # block -> (tile, b, nb) map for the flat pair loop
            blk2tile = []
            for t in range(TS):
                for b in range(counts[t]):
                    blk2tile.append((t, b, counts[t]))

            def _epilogue(t, acc):
                den = blkp.tile([P, H], f32, tag="den")
                nc.scalar.activation(den[:], acc[:, 0:8],
                                     mybir.ActivationFunctionType.Copy,
                                     bias=1e-9)
                rden = blkp.tile([P, H], f32, tag="rden")
                nc.vector.reciprocal(rden[:], den[:])
                aggs = blkp.tile([P, NKRON], bf16, tag="aggs")
                nc.vector.tensor_tensor(
                    out=aggs[:].rearrange("p (a b) -> p a b", b=NF12),
                    in0=acc[:, 8:NU].rearrange("p (a b) -> p a b", b=NF12),
                    in1=rden[:, :, None].to_broadcast([P, H, NF12]),
                    op=mybir.AluOpType.mult,
                )
                psT2 = psmiscp.tile([NKRON, P], bf16, tag="T2")
                nc.tensor.transpose(out=psT2[:], in_=aggs[:], identity=identb_sb[:])
                aggT = blkp.tile([NKRON + 8, P], bf16, tag="aggT")
                nc.scalar.copy(aggT[0:NKRON, :], psT2[:])
                nc.gpsimd.tensor_copy(aggT[NKRON:NKRON + 8, :],
                                       xT8c_sb[:, t * P:(t + 1) * P])
                pso = psmiscp.tile([P, DIM], f32, tag="T")
                nc.tensor.matmul(pso[:], lhsT=aggT[:], rhs=WblkWo_sb[:],
                                 start=True, stop=True)
                nc.scalar.activation(hall[:, t * DIM:(t + 1) * DIM], pso[:],
                                     mybir.ActivationFunctionType.Relu)
                if t % 4 == 3 or t == TS - 1:
                    s = t // 4
                    c0, c1 = s * 4 * DIM, (t + 1) * DIM
                    nc.tensor.matmul(psS[:, 0:c1 - c0], lhsT=ones_sb[:],
                                     rhs=hall[:, c0:c1],
                                     start=(s == 0), stop=(s == nslice - 1))

            acc_state = [None]

            def scatter_one(item, q):
                g0, rhswm, ohc, cb0 = item
                t, b, nb = blk2tile[g0 + q]
                if b == 0:
                    acc_state[0] = psaccp.tile([P, NU], f32, tag="acc",
                                               name="acc")
                acc = acc_state[0]
                nc.tensor.matmul(acc[:], lhsT=ohc[:, cb0 + q, :],
                                 rhs=rhswm[:, q, :],
                                 start=(b == 0), stop=(b == nb - 1))
                if b == nb - 1:
                    _epilogue(t, acc)

            pend = []
            for g in range(0, C, GB):
                ci = cidx_of[g]
                if g == bounds[ci] and ci not in chunks:
                    load_chunk(ci)
                st, ohc, cg0 = chunks[ci]
                cb = g - cg0

                # interleave this group's main matmuls 1:1 with the scatter
                # matmuls of the group issued two iterations ago: alternating
                # PSUM targets lets the PE pull the next weight-load ahead.
                sc = pend.pop(0) if len(pend) > 3 else None
                psm = psmainp.tile([P, GB, H], f32, tag="main")
                for q in range(GB):
                    nc.tensor.matmul(psm[:, q, :],
                                     lhsT=st[:, (cb + q) * P:(cb + q + 1) * P],
                                     rhs=rhs96_sb[:], start=True, stop=True)
                    if sc is not None:
                        scatter_one(sc, q)

                rhswm = blkp.tile([P, GB, NU], bf16, tag="rhswm")
                nc.scalar.activation(rhswm[:, :, 0:8], psm[:],
                                     mybir.ActivationFunctionType.Exp)
                nc.vector.tensor_tensor(
                    out=rhswm[:, :, 8:NU].rearrange("p c (a b) -> p c a b",
                                                    b=NF12),
                    in0=(se12a_sb[:, g:g + GB, None, :]
                         if g + GB <= SE_SPLIT else
                         se12b_sb[:, g - SE_SPLIT:g - SE_SPLIT + GB, None, :]
                         ).to_broadcast([P, GB, H, NF12]),
                    in1=rhswm[:, :, 0:8, None].to_broadcast([P, GB, H, NF12]),
                    op=mybir.AluOpType.mult,
                )
                pend.append((g, rhswm, ohc, cb))
            while pend:
                item = pend.pop(0)
                for q in range(GB):
                    scatter_one(item, q)

            Scopy = constp.tile([1, 4 * DIM], f32, tag="Scopy")
            nc.vector.tensor_copy(Scopy[:], psS[:])
            nc.sync.dma_start(out=S_out, in_=Scopy[:])

    nc.compile()
    return nc


def _bf16(a):
    import ml_dtypes
    return np.asarray(a).astype(ml_dtypes.bfloat16)


# ---------------------------------------------------------------- entry point
def kernel(**inputs):
    pos = np.asarray(inputs["pos"], np.float32)
    edge_attr = np.asarray(inputs["edge_attr"], np.float32)
    emb = np.asarray(inputs["emb"], np.float32)
    Wq = np.asarray(inputs["Wq"], np.float32)
    Wk = np.asarray(inputs["Wk"], np.float32)
    Wv = np.asarray(inputs["Wv"], np.float32)
    We = np.asarray(inputs["We"], np.float32)
    Wo = np.asarray(inputs["Wo"], np.float32)
    lin_w = np.asarray(inputs["lin_w"], np.float32)
    lin_b = np.asarray(inputs["lin_b"], np.float32)
    z = inputs["z"]
    edge_index = inputs["edge_index"]

    shared, percore, meta = _host_prep(pos, edge_attr, emb, Wq, Wk, Wv, We, Wo,
                                       z, edge_index)
    N = meta["N"]

    key = (tuple(meta["counts"]), meta["C"], meta["TS"], meta["npad"])
    nc = _PROG_CACHE.get(key)
    if nc is None:
        nc = _build_program(meta["counts"], meta["C"], meta["TS"], meta["npad"])
        _PROG_CACHE[key] = nc

    in_maps = []
    for j in range(N_CORES):
        m = {
            "rhs96": shared["rhs96"],
            "WblkWo": shared["WblkWo"],
            "ident": shared["ident"],
            "ones": shared["ones"],
            "srcfacT": percore["srcfacT"][j],
            "se12": percore["se12"][j],
            "ohmat": percore["ohmat"][j],
            "xT8c": percore["xT8c"][j],
        }
        in_maps.append(m)

    from concourse.bass_utils import run_bass_kernel_spmd
    res = run_bass_kernel_spmd(nc, in_maps, core_ids=list(range(N_CORES)),
                               trace=PROFILE, trace_cores=TRACE_CORES)
    global LAST_RESULT
    LAST_RESULT = res
    S = np.zeros(DIM, np.float64)
    for r in res.results:
        S += r["S_out"][0].astype(np.float64).reshape(4, DIM).sum(axis=0)
    y = (S.astype(np.float32) @ lin_w) / np.float32(N) + lin_b
    return y.reshape(1, 1).astype(np.float32)


# revision 36
# speedup vs baseline: 1.0631x; 1.0631x over previous
"""EquivariantGNN message-passing kernel for Trainium2 (8 NeuronCores, SPMD).

Math (matches the reference):
  x   = [pos | onehot(z)] @ [[I3,0],[0,emb]]          (rank-8 node features)
  logits[e,h] = 0.25 * q[dst]. (k[src]+eb)
              = sum_{i,j} x8[dst][i]*srcext[e][j] * Bvec[(i,j),h]
  w = exp(logits)  (no max subtraction needed; logits are O(10))
  den[n,h] = sum_{dst(e)=n} w
  U[n,(h,j)] = sum_{dst(e)=n} w[e,h]*srcext[e,j]      (96 values per node)
  agg[n,h,:] = (U[n,h,:]/den[n,h]) @ Wve12[:,h-slice]  (ve folded per NODE)
  out = agg @ Wo + x ; S = sum_n relu(out) ; answer = (S @ lin_w)/N + lin_b

Device strategy per core: edges sorted by dst, 128-edge blocks each fully inside
one 128-node tile.  Host ships, per edge, the 96-dim kron row
x8[dst] (x) srcext (lhsT layout), plus srcext row-major.  Per block one tiny
matmul (rhs [96,8]) yields the logits; ACT exponentiates; DVE forms
[w | w (x) srcext] (104 wide); a onehot(localdst) fp8 matmul
scatter-accumulates it into a per-tile PSUM accumulator.  The tile epilogue
normalizes U by den, transposes, and applies the host-folded
(Wve12 blockdiag) @ Wo plus the x8 @ (J8 @ ... residual), relu, and
accumulates the node-sum S via a ones-matmul.
"""

import math
import os
import sys

import numpy as np

for _p in ("/opt/trn_rl_repo", "/root/.axon_site/_ro/trn_rl_repo"):
    if os.path.isdir(_p) and _p not in sys.path:
        sys.path.insert(0, _p)

P = 128
DIM = 128
H, DH = 8, 16
DE = 4
NF12 = 12   # srcext = [ea(4) | x8[src](8)]
NKRON = 96  # U payload width: H * NF12
NLG = 44    # logit lhsT rows: pos-kron(36) + dst-type term(8)
NU = 8 + NKRON  # scatter payload: [w(8) | w (x) srcext (96)]
N_CORES = 8
GB = 8       # blocks per processing group
GCHUNK = 32  # blocks per gather chunk

# test-harness knobs (the grading harness just calls kernel() with defaults)
PROFILE = False
TRACE_CORES = None
LAST_RESULT = None  # BassKernelResults of the last run (for profiling)
_PROG_CACHE = {}


# ---------------------------------------------------------------- host prep
def _host_prep(pos, edge_attr, emb, Wq, Wk, Wv, We, Wo, z, edge_index):
    f32 = np.float32
    N = pos.shape[0]
    NT = emb.shape[0]
    ntiles = (N + P - 1) // P
    npad = ntiles * P

    z = np.asarray(z).astype(np.int64)
    src = np.asarray(edge_index[0]).astype(np.int64)
    dst = np.asarray(edge_index[1]).astype(np.int64)
    E = src.shape[0]

    onehot = np.zeros((N, NT), f32)
    onehot[np.arange(N), z] = 1.0
    x8 = np.concatenate([np.asarray(pos, f32), onehot], axis=1)  # [N, 8]
    x8p = np.zeros((npad, 8), f32)
    x8p[:N] = x8

    # rank-8 weight factors
    Wq8 = np.vstack([Wq[:3], emb @ Wq[3:]]).astype(f32)  # [8,128]
    Wk8 = np.vstack([Wk[:3], emb @ Wk[3:]]).astype(f32)
    Wv8 = np.vstack([Wv[:3], emb @ Wv[3:]]).astype(f32)
    # srcext rows = [ea(4); x8src(8)]:  ke = srcext @ [[We],[Wk8]]
    Wke12 = np.vstack([We, Wk8]).astype(f32)   # [12,128]
    Wve12 = np.vstack([We, Wv8]).astype(f32)   # [12,128]

    # bilinear logits: logits[e,h] = sum_{i,j} x8dst[i]*srcext[j]*Bvec[(i,j),h]
    Bvec = np.zeros((NKRON, H), f32)
    for h in range(H):
        Bh = 0.25 * (Wq8[:, h * DH:(h + 1) * DH]
                     @ Wke12[:, h * DH:(h + 1) * DH].T)  # [8,12]
        Bvec[:, h] = Bh.reshape(NKRON)

    # U[(h,j)] -> out:  WblkWo[(h,j), d'] = sum_d Wve12[j, h*16+d] * Wo[h*16+d, d']
    Wo32 = np.asarray(Wo, f32)
    WblkWo = np.zeros((NKRON, DIM), f32)
    for h in range(H):
        WblkWo[h * NF12:(h + 1) * NF12] = (
            Wve12[:, h * DH:(h + 1) * DH] @ Wo32[h * DH:(h + 1) * DH])

    J8 = np.zeros((8, DIM), f32)  # x = x8 @ J8
    J8[0:3, 0:3] = np.eye(3, dtype=f32)
    J8[3:8, 3:DIM] = emb

    # ---- sort edges by dst, split into per-node-tile runs
    perm = np.argsort(dst, kind="stable")
    src_s, dst_s = src[perm], dst[perm]
    ea_s = np.asarray(edge_attr, f32)[perm]
    tile_of_edge = dst_s // P
    starts = np.searchsorted(tile_of_edge, np.arange(ntiles))
    ends = np.searchsorted(tile_of_edge, np.arange(ntiles) + 1)
    ecnt = ends - starts
    nb = np.maximum(1, (ecnt + P - 1) // P)  # blocks per real tile

    # per-edge srcext + kron rows
    sext = np.empty((E, NF12), f32)
    sext[:, 0:DE] = ea_s
    sext[:, DE:NF12] = x8[src_s]
    kron = (x8[dst_s][:, :, None] * sext[:, None, :]).reshape(E, NKRON)

    # ---- uniform schedule across cores: pad tile list to multiple of 8,
    # sort by block count desc, deal groups of 8 (one tile per core),
    # pad each group to the group max -> identical counts on every core.
    ntiles_tot = ((ntiles + N_CORES - 1) // N_CORES) * N_CORES
    nb_all = np.concatenate([nb, np.ones(ntiles_tot - ntiles, np.int64)])
    order = np.argsort(-nb_all, kind="stable")
    TS = ntiles_tot // N_CORES  # tiles per core
    counts = [int(nb_all[order[8 * k]]) for k in range(TS)]  # group max
    counts[-1] += (-sum(counts)) % GB  # block count multiple of the group size
    C = int(sum(counts))

    import ml_dtypes

    srcfac = np.zeros((N_CORES, C, P, NKRON), ml_dtypes.bfloat16)
    se12 = np.zeros((N_CORES, P, C, NF12), ml_dtypes.bfloat16)
    ohmat = np.zeros((N_CORES, C, P, P), ml_dtypes.float8_e4m3fn)  # onehot(localdst)
    xT8c = np.zeros((N_CORES, 8, TS * P), f32)

    offs = np.concatenate([[0], np.cumsum(counts)])
    for k in range(TS):
        for j in range(N_CORES):
            t = int(order[8 * k + j])
            if t >= ntiles:
                continue  # dummy tile: all-dummy blocks, zero xT8c
            xT8c[j, :, k * P:(k + 1) * P] = x8p[t * P:(t + 1) * P].T
            e0, e1 = int(starts[t]), int(ends[t])
            ne = e1 - e0
            if ne == 0:
                continue
            c0 = int(offs[k])
            flat = np.arange(ne)
            cc = c0 + flat // P
            pp = flat % P
            srcfac[j, cc, pp, :] = kron[e0:e1]
            se12[j, pp, cc, :] = sext[e0:e1]
            ohmat[j, cc, pp, dst_s[e0:e1] - t * P] = 1.0

    ident = np.eye(P, dtype=f32)
    ones = np.ones((P, 1), f32)

    # device layouts
    srcfacT = np.ascontiguousarray(
        srcfac.transpose(0, 3, 1, 2)).reshape(N_CORES, NKRON, C * P)
    ohmatd = np.ascontiguousarray(ohmat.transpose(0, 2, 1, 3))  # [j, P, C, P]

    WblkWoJ = np.vstack([WblkWo, J8])  # residual folded as 8 extra lhsT rows
    shared = dict(rhs96=Bvec.astype(ml_dtypes.bfloat16),
                  WblkWo=WblkWoJ.astype(ml_dtypes.bfloat16), ident=ident,
                  ones=ones.astype(ml_dtypes.bfloat16))
    percore = dict(srcfacT=srcfacT, se12=se12, ohmat=ohmatd,
                   xT8c=xT8c.astype(ml_dtypes.bfloat16))
    meta = dict(counts=counts, C=C, TS=TS, npad=npad, N=N, E=E)
    return shared, percore, meta


# ---------------------------------------------------------------- device code
def _build_program(counts, C, TS, npad):
    import concourse.bacc as bacc
    import concourse.bass as bass
    import concourse.tile as tile
    from concourse import mybir
    from concourse._compat import with_exitstack  # noqa: F401

    f32 = mybir.dt.float32
    bf16 = mybir.dt.bfloat16
    fp8 = mybir.dt.float8e4

    nc = bacc.Bacc("TRN2", target_bir_lowering=False, debug=False,
                   enable_asserts=False, num_devices=N_CORES)

    srcfacT_in = nc.dram_tensor("srcfacT", [NKRON, C * P], bf16,
                                kind="ExternalInput").ap()
    se12_in = nc.dram_tensor("se12", [P, C, NF12], bf16,
                             kind="ExternalInput").ap()
    ohmat_in = nc.dram_tensor("ohmat", [P, C, P], fp8, kind="ExternalInput").ap()
    xT8c_in = nc.dram_tensor("xT8c", [8, TS * P], bf16, kind="ExternalInput").ap()
    rhs96_in = nc.dram_tensor("rhs96", [NKRON, H], bf16,
                              kind="ExternalInput").ap()
    WblkWo_in = nc.dram_tensor("WblkWo", [NKRON + 8, DIM], bf16,
                               kind="ExternalInput").ap()
    ident_in = nc.dram_tensor("ident", [P, P], f32, kind="ExternalInput").ap()
    ones_in = nc.dram_tensor("ones", [P, 1], bf16, kind="ExternalInput").ap()
    S_out = nc.dram_tensor("S_out", [1, 4 * DIM], f32, kind="ExternalOutput").ap()

    with tile.TileContext(nc) as tc:
        with (
            tc.tile_pool(name="const", bufs=1) as constp,
            tc.tile_pool(name="chunks", bufs=6) as chunkp,
            tc.tile_pool(name="blk", bufs=6) as blkp,
            tc.tile_pool(name="psmain", bufs=2, space="PSUM") as psmainp,
            tc.tile_pool(name="psmisc", bufs=1, space="PSUM") as psmiscp,
            tc.tile_pool(name="psacc", bufs=3, space="PSUM") as psaccp,
            tc.tile_pool(name="psS", bufs=1, space="PSUM") as psSp,
        ):
            # HAM warmup: ~5us of back-to-back matmuls on a memset tile (no
            # DMA dependency) so the PE clock un-throttles to 2.4 GHz while
            # the first chunks stream in.
            warm_sb = constp.tile([P, 2 * P], bf16, tag="warmsrc")
            nc.gpsimd.memset(warm_sb[:], 0.5)
            pswarm = psmiscp.tile([P, DIM], f32, tag="T")
            for _ in range(16):
                nc.tensor.matmul(pswarm[:], lhsT=warm_sb[:, 0:P],
                                 rhs=warm_sb[:, P:2 * P], start=True, stop=True)

            # chunk schedule: small prologue chunks so compute starts early
            bounds = [0]
            for nxt in (8, 24, 56):
                if nxt < C:
                    bounds.append(nxt)
            while bounds[-1] + GCHUNK < C:
                bounds.append(bounds[-1] + GCHUNK)
            bounds.append(C)
            cidx_of = {}
            for ci in range(len(bounds) - 1):
                for g in range(bounds[ci], bounds[ci + 1]):
                    cidx_of[g] = ci

            chunks = {}

            def load_chunk(ci):
                g0, g1 = bounds[ci], bounds[ci + 1]
                gn = g1 - g0
                st = chunkp.tile([NKRON, GCHUNK * P], bf16, tag="srcT")
                ohc = chunkp.tile([P, GCHUNK, P], fp8, tag="ohc")
                nc.sync.dma_start(out=st[:, :gn * P],
                                  in_=srcfacT_in[:, g0 * P:(g0 + gn) * P])
                nc.sync.dma_start(out=ohc[:, :gn, :],
                                  in_=ohmat_in[:, g0:g0 + gn, :])
                chunks[ci] = (st, ohc, g0)

            # critical-path loads first: the logit weights, srcext rows and
            # the first two edge chunks; remaining constants go out on the
            # (otherwise idle) GpSimd DMA queue.
            load_chunk(0)
            rhs96_sb = constp.tile_from(rhs96_in)
            SE_SPLIT = min(56, C)
            se12a_sb = constp.tile([P, SE_SPLIT, NF12], bf16, tag="se12a")
            nc.sync.dma_start(out=se12a_sb[:], in_=se12_in[:, 0:SE_SPLIT, :])
            load_chunk(1)
            if len(bounds) > 3:
                load_chunk(2)
            se12b_sb = constp.tile([P, C - SE_SPLIT, NF12], bf16, tag="se12b")
            nc.sync.dma_start(out=se12b_sb[:], in_=se12_in[:, SE_SPLIT:C, :])
            pool_eng = mybir.EngineType.Pool
            WblkWo_sb = constp.tile_from(WblkWo_in, forced_dma_engine=pool_eng)
            xT8c_sb = constp.tile_from(xT8c_in, forced_dma_engine=pool_eng)
            ident_sb = constp.tile_from(ident_in, forced_dma_engine=pool_eng)
            identb_sb = constp.tile_from(ident_in, dtype=mybir.dt.bfloat16,
                                         forced_dma_engine=pool_eng,
                                         force_copy=True)
            ones_sb = constp.tile_from(ones_in, forced_dma_engine=pool_eng)

            psS = psSp.tile([1, 4 * DIM], f32, tag="S")
            hall = constp.tile([P, TS * DIM], bf16, tag="hall")
            nslice = (TS + 3) // 4

            # block -> (tile, b, nb) map for the flat pair loop
            blk2tile = []
            for t in range(TS):
                for b in range(counts[t]):
                    blk2tile.append((t, b, counts[t]))

            def _epilogue(t, acc):
                den = blkp.tile([P, H], f32, tag="den")
                nc.scalar.activation(den[:], acc[:, 0:8],
                                     mybir.ActivationFunctionType.Copy,
                                     bias=1e-9)
                rden = blkp.tile([P, H], f32, tag="rden")
                nc.vector.reciprocal(rden[:], den[:])
                aggs = blkp.tile([P, NKRON], bf16, tag="aggs")
                nc.vector.tensor_tensor(
                    out=aggs[:].rearrange("p (a b) -> p a b", b=NF12),
                    in0=acc[:, 8:NU].rearrange("p (a b) -> p a b", b=NF12),
                    in1=rden[:, :, None].to_broadcast([P, H, NF12]),
                    op=mybir.AluOpType.mult,
                )
                psT2 = psmiscp.tile([NKRON, P], bf16, tag="T2")
                nc.tensor.transpose(out=psT2[:], in_=aggs[:], identity=identb_sb[:])
                aggT = blkp.tile([NKRON + 8, P], bf16, tag="aggT")
                nc.scalar.copy(aggT[0:NKRON, :], psT2[:])
                nc.gpsimd.tensor_copy(aggT[NKRON:NKRON + 8, :],
                                       xT8c_sb[:, t * P:(t + 1) * P])
                pso = psmiscp.tile([P, DIM], f32, tag="T")
                nc.tensor.matmul(pso[:], lhsT=aggT[:], rhs=WblkWo_sb[:],
                                 start=True, stop=True)
                nc.scalar.activation(hall[:, t * DIM:(t + 1) * DIM], pso[:],
                                     mybir.ActivationFunctionType.Relu)
                if t % 4 == 3 or t == TS - 1:
                    s = t // 4
                    c0, c1 = s * 4 * DIM, (t + 1) * DIM
                    nc.tensor.matmul(psS[:, 0:c1 - c0], lhsT=ones_sb[:],
                                     rhs=hall[:, c0:c1],
                                     start=(s == 0), stop=(s == nslice - 1))

            acc_state = [None]

            def scatter_one(item, q):
                g0, rhswm, ohc, cb0 = item
                t, b, nb = blk2tile[g0 + q]
                if b == 0:
                    acc_state[0] = psaccp.tile([P, NU], f32, tag="acc",
                                               name="acc")
                acc = acc_state[0]
                nc.tensor.matmul(acc[:], lhsT=ohc[:, cb0 + q, :],
                                 rhs=rhswm[:, q, :],
                                 start=(b == 0), stop=(b == nb - 1))
                if b == nb - 1:
                    _epilogue(t, acc)

            pend = []
            for g in range(0, C, GB):
                ci = cidx_of[g]
                if g == bounds[ci] and ci not in chunks:
                    load_chunk(ci)
                st, ohc, cg0 = chunks[ci]
                cb = g - cg0

                # interleave this group's main matmuls 1:1 with the scatter
                # matmuls of the group issued two iterations ago: alternating
                # PSUM targets lets the PE pull the next weight-load ahead.
                sc = pend.pop(0) if len(pend) > 2 else None
                psm = psmainp.tile([P, GB, H], f32, tag="main")
                for q in range(GB):
                    nc.tensor.matmul(psm[:, q, :],
                                     lhsT=st[:, (cb + q) * P:(cb + q + 1) * P],
                                     rhs=rhs96_sb[:], start=True, stop=True)
                    if sc is not None:
                        scatter_one(sc, q)

                rhswm = blkp.tile([P, GB, NU], bf16, tag="rhswm")
                nc.scalar.activation(rhswm[:, :, 0:8], psm[:],
                                     mybir.ActivationFunctionType.Exp)
                nc.vector.tensor_tensor(
                    out=rhswm[:, :, 8:NU].rearrange("p c (a b) -> p c a b",
                                                    b=NF12),
                    in0=(se12a_sb[:, g:g + GB, None, :]
                         if g + GB <= SE_SPLIT else
                         se12b_sb[:, g - SE_SPLIT:g - SE_SPLIT + GB, None, :]
                         ).to_broadcast([P, GB, H, NF12]),
                    in1=rhswm[:, :, 0:8, None].to_broadcast([P, GB, H, NF12]),
                    op=mybir.AluOpType.mult,
                )
                pend.append((g, rhswm, ohc, cb))
            while pend:
                item = pend.pop(0)
                for q in range(GB):
                    scatter_one(item, q)

            Scopy = constp.tile([1, 4 * DIM], f32, tag="Scopy")
            nc.vector.tensor_copy(Scopy[:], psS[:])
            nc.sync.dma_start(out=S_out, in_=Scopy[:])

    nc.compile()
    return nc


def _bf16(a):
    import ml_dtypes
    return np.asarray(a).astype(ml_dtypes.bfloat16)


# ---------------------------------------------------------------- entry point
def kernel(**inputs):
    pos = np.asarray(inputs["pos"], np.float32)
    edge_attr = np.asarray(inputs["edge_attr"], np.float32)
    emb = np.asarray(inputs["emb"], np.float32)
    Wq = np.asarray(inputs["Wq"], np.float32)
    Wk = np.asarray(inputs["Wk"], np.float32)
    Wv = np.asarray(inputs["Wv"], np.float32)
    We = np.asarray(inputs["We"], np.float32)
    Wo = np.asarray(inputs["Wo"], np.float32)
    lin_w = np.asarray(inputs["lin_w"], np.float32)
    lin_b = np.asarray(inputs["lin_b"], np.float32)
    z = inputs["z"]
    edge_index = inputs["edge_index"]

    shared, percore, meta = _host_prep(pos, edge_attr, emb, Wq, Wk, Wv, We, Wo,
                                       z, edge_index)
    N = meta["N"]

    key = (tuple(meta["counts"]), meta["C"], meta["TS"], meta["npad"])
    nc = _PROG_CACHE.get(key)
    if nc is None:
        nc = _build_program(meta["counts"], meta["C"], meta["TS"], meta["npad"])
        _PROG_CACHE[key] = nc

    in_maps = []
    for j in range(N_CORES):
        m = {
            "rhs96": shared["rhs96"],
            "WblkWo": shared["WblkWo"],
            "ident": shared["ident"],
            "ones": shared["ones"],
            "srcfacT": percore["srcfacT"][j],
            "se12": percore["se12"][j],
            "ohmat": percore["ohmat"][j],
            "xT8c": percore["xT8c"][j],
        }
        in_maps.append(m)

    from concourse.bass_utils import run_bass_kernel_spmd
    res = run_bass_kernel_spmd(nc, in_maps, core_ids=list(range(N_CORES)),
                               trace=PROFILE, trace_cores=TRACE_CORES)
    global LAST_RESULT
    LAST_RESULT = res
    S = np.zeros(DIM, np.float64)
    for r in res.results:
        S += r["S_out"][0].astype(np.float64).reshape(4, DIM).sum(axis=0)
    y = (S.astype(np.float32) @ lin_w) / np.float32(N) + lin_b
    return y.reshape(1, 1).astype(np.float32)
